# revision 21
# baseline (speedup 1.0000x reference)
"""Trainium2 Bass kernel for the moe_routing classifier problem.

Computation (per batch row b, class c):
  cos[b,c,s]  = cosine(emb[b], weight[c,s])            (64 sub-prototypes)
  top-8 over s, softmax weights w, protos = sum_k w_k * weight[c, idx_k]
  out[b,c]    = ((1 + cosine(protos, emb[b])) / 2 + 1e-8) / 0.1

Key algebra used by the kernel (avoids gathers entirely):
  E[b,c,s]   = exp(score) masked to the top-8 entries (unnormalized softmax)
  dot2*Z     = sum_s E * dot_raw                        (Z cancels later)
  |protos|^2*Z^2 = E^T (W W^T) E  via per-class Gram matrices
  cos2       = (sum_s E*dot_raw) * inv|emb| / sqrt(E^T G E)

Sharding: classes are split across the 8 cores (32 classes each); emb is
replicated. Each core writes a [1024, 32] slice of the output.

Engine schedule: per batch tile, stage A (matmul -> exp -> top-8 mask) and
stage B (pair-transpose E -> EG matmul -> reductions) are emitted with a
one-tile skew so each engine's in-order stream never stalls on the
cross-engine chain of the same tile.
"""

import numpy as np

B, D, C, S = 1024, 128, 256, 64
NCORES = 8
C_LOC = C // NCORES        # 32 classes per core
CS = C_LOC * S             # 2048 anchor rows per core
P = 128                    # partitions
NBT = B // P               # 8 batch tiles
NWT = CS // P              # 16 weight tiles
EPS = 1e-8
SC_BIAS = 0.5 + EPS        # score = 0.5*cos + SC_BIAS
OUT_SCALE = 5.0            # ((1+x)/2 + 1e-8) / 0.1 = 5x + 5 + 1e-7
OUT_BIAS = 5.0 + 1e-7

_CACHE = {}


def build_nc():
    import concourse.bass as bass
    import concourse.tile as tile
    from concourse import bacc, mybir
    from concourse.masks import make_identity
    from contextlib import ExitStack

    f32 = mybir.dt.float32
    AF = mybir.ActivationFunctionType
    ALU = mybir.AluOpType

    nc = bacc.Bacc(None, target_bir_lowering=False)
    emb_d = nc.dram_tensor("emb", [B, D], f32, kind="ExternalInput")
    w_d = nc.dram_tensor("weight", [CS, D], f32, kind="ExternalInput")
    out_d = nc.dram_tensor("out", [B, C_LOC], f32, kind="ExternalOutput")

    with tile.TileContext(nc) as tc, ExitStack() as ctx:
        sing = ctx.enter_context(tc.tile_pool(name="sing", bufs=1))
        dram = ctx.enter_context(tc.tile_pool(name="dram", bufs=1, space="DRAM"))
        work = ctx.enter_context(tc.tile_pool(name="work", bufs=3))
        small = ctx.enter_context(tc.tile_pool(name="small", bufs=4))
        jk = ctx.enter_context(tc.tile_pool(name="jk", bufs=8))
        fpool = ctx.enter_context(tc.tile_pool(name="fpool", bufs=4))
        ps_mm = ctx.enter_context(tc.tile_pool(name="ps_mm", bufs=2, space="PSUM"))
        ps_tr = ctx.enter_context(tc.tile_pool(name="ps_tr", bufs=3, space="PSUM"))
        ps_eg = ctx.enter_context(tc.tile_pool(name="ps_eg", bufs=3, space="PSUM"))

        ident = sing.tile([P, P], f32)
        make_identity(nc, ident[:])
        sbias = sing.tile([P, 1], f32)     # score bias as AP (Exp needs AP bias)
        nc.vector.memset(sbias[:], SC_BIAS)

        # ---------------- load inputs (emb first, separate DMA queues) ----
        En = sing.tile([P, NBT, D], f32)   # emb rows, tiled by 128
        nc.sync.dma_start(En[:], emb_d[:].rearrange("(t p) d -> p t d", p=P))
        Wn = sing.tile([P, NWT, D], f32)   # weight rows, tiled by 128
        nc.gpsimd.dma_start(Wn[:], w_d[:].rearrange("(t p) d -> p t d", p=P))

        # ---------------- norms (emb before weights) ----------------
        esq = sing.tile([P, NBT], f32)
        for t in range(NBT):
            j = jk.tile([P, D], f32, tag="jact")
            nc.scalar.activation(j[:], En[:, t], AF.Square,
                                 accum_out=esq[:, t : t + 1])
        ne = sing.tile([P, NBT], f32)      # ||emb||
        nc.scalar.activation(ne[:], esq[:], AF.Sqrt)
        ine = sing.tile([P, NBT], f32)     # 1/||emb||
        hine = sing.tile([P, NBT], f32)    # 0.5/||emb||
        nc.vector.reciprocal_approx_accurate(ine[:], ne[:], hine[:])
        nc.vector.tensor_scalar_mul(hine[:], ine[:], 0.5)
        nwsq = sing.tile([P, NWT], f32)    # ||w_row||^2, row-tiled layout
        for t in range(NWT):
            j = jk.tile([P, D], f32, tag="jact")
            nc.scalar.activation(j[:], Wn[:, t], AF.Square,
                                 accum_out=nwsq[:, t : t + 1])

        # nw broadcast rows: roundtrip through DRAM to reorder + partition-bcast
        scr = dram.tile([CS], f32)
        nc.sync.dma_start(scr[:].rearrange("(t p) -> p t", p=P), nwsq[:])
        scr_bc = bass.AP(
            tensor=scr[:].tensor, offset=scr[:].offset,
            ap=[[0, P]] + list(scr[:].ap),
        )
        NWB = sing.tile([P, CS], f32)      # ||w_row|| broadcast over partitions
        nc.sync.dma_start(NWB[:], scr_bc)
        nc.scalar.activation(NWB[:], NWB[:], AF.Sqrt)

        # ---------------- transposed operands ----------------
        # normalize anchor rows first (per-partition scale), then transpose,
        # so VT chunks become available early for the first batch tile.
        nw_row = sing.tile([P, NWT], f32)   # ||w_row||, row-tiled
        inw_row = sing.tile([P, NWT], f32)  # 1/||w_row||
        inw_scr = sing.tile([P, NWT], f32)
        nc.scalar.activation(nw_row[:], nwsq[:], AF.Sqrt)
        nc.vector.reciprocal_approx_accurate(inw_row[:], nw_row[:], inw_scr[:])

        embT = sing.tile([P, B], f32)      # emb^T [d, b]
        for t in range(NBT):
            pst = ps_tr.tile([P, 2 * P], f32, tag="tr")
            nc.tensor.transpose(pst[:, :P], En[:, t], ident[:])
            nc.scalar.copy(embT[:, t * P : (t + 1) * P], pst[:, :P])

        VT = sing.tile([P, CS], f32)       # normalized anchors transposed
        Vn = sing.tile([P, NWT, D], f32)
        for t in range(NWT):
            nc.vector.tensor_scalar_mul(Vn[:, t], Wn[:, t],
                                        inw_row[:, t : t + 1])
            pst = ps_tr.tile([P, 2 * P], f32, tag="tr")
            nc.tensor.transpose(pst[:, :P], Vn[:, t], ident[:])
            nc.scalar.copy(VT[:, t * P : (t + 1) * P], pst[:, :P])



        # persistent per-tile outputs for the batched tail
        d2zall = sing.tile([P, NBT, C_LOC], f32)
        np2zall = sing.tile([P, NBT, C_LOC], f32)

        def build_gram():
            # per-class raw Gram matrices G_c = W_c W_c^T [64, 64], packed
            # into block-diagonal pair matrices GP[:, q*128:(q+1)*128] =
            # [[G_2q, 0], [0, G_2q+1]] so one full-size (0,0)-quadrant matmul
            # computes EG for a transposed class pair (quadrant matmuls
            # interleaved with transposes crash the device).
            WT = sing.tile([P, CS], f32)   # raw W^T = VT * ||w||, gram only
            nc.vector.tensor_mul(WT[:], VT[:], NWB[:])
            Gtmp = sing.tile([S, CS], f32)
            for c in range(C_LOC):
                cs = slice(c * S, (c + 1) * S)
                psg = ps_tr.tile([P, 2 * P], f32, tag="tr")
                nc.tensor.matmul(psg[:S, :S], WT[:, cs], WT[:, cs])
                nc.scalar.copy(Gtmp[:, cs], psg[:S, :S])
            GP = sing.tile([P, CS], f32)
            nc.vector.memset(GP[:], 0.0)
            gt3 = Gtmp[:].rearrange("p (q j) -> p q j", j=2 * S)
            gp3 = GP[:].rearrange("p (q j) -> p q j", j=2 * S)
            nc.sync.dma_start(gp3[0:S, :, 0:S], gt3[:, :, 0:S])
            nc.sync.dma_start(gp3[S : 2 * S, :, S : 2 * S], gt3[:, :, S : 2 * S])
            return GP

        tiles = {}

        def stageA(bt):
            bsl = slice(bt * P, (bt + 1) * P)
            exps = work.tile([P, CS], f32, tag="exps", bufs=2)
            dotr = work.tile([P, CS], f32, tag="dotr", bufs=2)
            for j in range(CS // 512):
                js = slice(j * 512, (j + 1) * 512)
                dotn = ps_mm.tile([P, 512], f32, tag="mm")
                nc.tensor.matmul(dotn[:], embT[:, bsl], VT[:, js])
                nc.scalar.activation(
                    exps[:, js], dotn[:], AF.Exp,
                    bias=sbias[:], scale=hine[:, bt : bt + 1],
                )
                nc.vector.tensor_mul(dotr[:, js], dotn[:], NWB[:, js])

            # top-8 selection per class: R = exps with top8 zeroed
            R = work.tile([P, CS], f32, tag="R", bufs=2)
            for c in range(C_LOC):
                cs = slice(c * S, (c + 1) * S)
                mx8 = small.tile([P, 8], f32, tag="mx8")
                nc.vector.max(out=mx8[:], in_=exps[:, cs])
                nc.vector.match_replace(
                    out=R[:, cs], in_to_replace=mx8[:],
                    in_values=exps[:, cs], imm_value=0.0,
                )
            E = work.tile([P, CS], f32, tag="E", bufs=5)
            nc.gpsimd.tensor_sub(E[:], exps[:], R[:])
            prod_d = work.tile([P, CS], f32, tag="pd", bufs=2)
            nc.gpsimd.tensor_mul(prod_d[:], E[:], dotr[:])
            tiles[bt] = (E, prod_d)

        def stageB(bt, GP):
            E, prod_d = tiles.pop(bt)
            nc.vector.tensor_reduce(
                d2zall[:, bt], prod_d[:].rearrange("p (c s) -> p c s", c=C_LOC),
                axis=mybir.AxisListType.X, op=ALU.add)
            prod_n = work.tile([P, CS], f32, tag="prod_n", bufs=2)
            for q8 in range(CS // 512):
                pse = ps_eg.tile([P, 512], f32, tag="eg")
                for qq in range(2):
                    pst = ps_tr.tile([P, 2 * P], f32, tag="tr")
                    Fq = fpool.tile([P, 2 * P], f32, tag="F")
                    for h in range(2):
                        q = 4 * q8 + 2 * qq + h
                        qs = slice(q * 128, (q + 1) * 128)
                        nc.tensor.transpose(
                            pst[:, h * 128 : (h + 1) * 128], E[:, qs], ident[:]
                        )
                    nc.scalar.copy(Fq[:], pst[:])
                    for h in range(2):
                        q = 4 * q8 + 2 * qq + h
                        qs = slice(q * 128, (q + 1) * 128)
                        nc.tensor.matmul(
                            pse[:, (2 * qq + h) * 128 : (2 * qq + h + 1) * 128],
                            Fq[:, h * 128 : (h + 1) * 128], GP[:, qs],
                        )
                nc.vector.tensor_mul(
                    prod_n[:, q8 * 512 : (q8 + 1) * 512],
                    pse[:], E[:, q8 * 512 : (q8 + 1) * 512],
                )
            nc.vector.tensor_reduce(
                np2zall[:, bt], prod_n[:].rearrange("p (c s) -> p c s", c=C_LOC),
                axis=mybir.AxisListType.X, op=ALU.add)

        # ---------------- software-pipelined main loop (skew 3) ----------
        stageA(0)
        stageA(1)
        GP = build_gram()
        stageA(2)
        stageA(3)
        for bt in range(4, NBT):
            stageA(bt)
            stageB(bt - 4, GP)
        for bt in range(NBT - 4, NBT):
            stageB(bt, GP)

        # ---------------- batched tail ----------------
        # cos2 = d2z * ine / sqrt(np2z);  out = 5*cos2 + 5 + 1e-7
        nps = sing.tile([P, NBT, C_LOC], f32)
        nc.scalar.activation(nps[:], np2zall[:], AF.Sqrt)
        rnp = sing.tile([P, NBT, C_LOC], f32)
        c2 = sing.tile([P, NBT, C_LOC], f32)
        nc.vector.reciprocal_approx_accurate(rnp[:], nps[:], c2[:])
        nc.vector.tensor_mul(c2[:], d2zall[:], rnp[:])
        ine_b = ine[:, :, None].to_broadcast([P, NBT, C_LOC])
        nc.vector.tensor_mul(c2[:], c2[:], ine_b)
        osb = sing.tile([P, NBT, C_LOC], f32)
        nc.vector.tensor_scalar(
            osb[:], c2[:], OUT_SCALE, OUT_BIAS, op0=ALU.mult, op1=ALU.add)
        nc.sync.dma_start(out_d[:].rearrange("(t p) c -> p t c", p=P), osb[:])

    nc.compile()
    return nc


def _get_nc():
    if "nc" not in _CACHE:
        _CACHE["nc"] = build_nc()
    return _CACHE["nc"]


def kernel(emb: np.ndarray, weight: np.ndarray) -> np.ndarray:
    from concourse.bass_utils import run_bass_kernel_spmd

    emb = np.ascontiguousarray(np.asarray(emb, dtype=np.float32))
    weight = np.ascontiguousarray(np.asarray(weight, dtype=np.float32))
    assert emb.shape == (B, D) and weight.shape == (C, S, D)

    nc = _get_nc()
    in_maps = [
        {
            "emb": emb,
            "weight": np.ascontiguousarray(
                weight[i * C_LOC : (i + 1) * C_LOC].reshape(CS, D)
            ),
        }
        for i in range(NCORES)
    ]
    res = run_bass_kernel_spmd(nc, in_maps, core_ids=list(range(NCORES)))
    return np.concatenate(
        [res.results[i]["out"] for i in range(NCORES)], axis=1
    )


# revision 22
# speedup vs baseline: 1.1333x; 1.1333x over previous
"""Trainium2 Bass kernel for the moe_routing classifier problem.

Computation (per batch row b, class c):
  cos[b,c,s]  = cosine(emb[b], weight[c,s])            (64 sub-prototypes)
  top-8 over s, softmax weights w, protos = sum_k w_k * weight[c, idx_k]
  out[b,c]    = ((1 + cosine(protos, emb[b])) / 2 + 1e-8) / 0.1

Key algebra used by the kernel (avoids gathers entirely):
  E[b,c,s]   = exp(score) masked to the top-8 entries (unnormalized softmax)
  dot2*Z     = sum_s E * dot_raw                        (Z cancels later)
  |protos|^2*Z^2 = E^T (W W^T) E  via per-class Gram matrices
  cos2       = (sum_s E*dot_raw) * inv|emb| / sqrt(E^T G E)

Sharding: classes are split across the 8 cores (32 classes each); emb is
replicated. Each core writes a [1024, 32] slice of the output.

Engine schedule: per batch tile, stage A (matmul -> exp -> top-8 mask) and
stage B (pair-transpose E -> EG matmul -> reductions) are emitted with a
one-tile skew so each engine's in-order stream never stalls on the
cross-engine chain of the same tile.
"""

import numpy as np

B, D, C, S = 1024, 128, 256, 64
NCORES = 8
C_LOC = C // NCORES        # 32 classes per core
CS = C_LOC * S             # 2048 anchor rows per core
P = 128                    # partitions
NBT = B // P               # 8 batch tiles
NWT = CS // P              # 16 weight tiles
EPS = 1e-8
SC_BIAS = 0.5 + EPS        # score = 0.5*cos + SC_BIAS
OUT_SCALE = 5.0            # ((1+x)/2 + 1e-8) / 0.1 = 5x + 5 + 1e-7
OUT_BIAS = 5.0 + 1e-7

_CACHE = {}


def build_nc():
    import concourse.bass as bass
    import concourse.tile as tile
    from concourse import bacc, mybir
    from concourse.masks import make_identity
    from contextlib import ExitStack

    f32 = mybir.dt.float32
    AF = mybir.ActivationFunctionType
    ALU = mybir.AluOpType

    nc = bacc.Bacc(None, target_bir_lowering=False)
    emb_d = nc.dram_tensor("emb", [B, D], f32, kind="ExternalInput")
    w_d = nc.dram_tensor("weight", [CS, D], f32, kind="ExternalInput")
    out_d = nc.dram_tensor("out", [B, C_LOC], f32, kind="ExternalOutput")

    with tile.TileContext(nc) as tc, ExitStack() as ctx:
        sing = ctx.enter_context(tc.tile_pool(name="sing", bufs=1))
        dram = ctx.enter_context(tc.tile_pool(name="dram", bufs=1, space="DRAM"))
        work = ctx.enter_context(tc.tile_pool(name="work", bufs=3))
        small = ctx.enter_context(tc.tile_pool(name="small", bufs=4))
        jk = ctx.enter_context(tc.tile_pool(name="jk", bufs=8))
        fpool = ctx.enter_context(tc.tile_pool(name="fpool", bufs=4))
        ps_mm = ctx.enter_context(tc.tile_pool(name="ps_mm", bufs=2, space="PSUM"))
        ps_tr = ctx.enter_context(tc.tile_pool(name="ps_tr", bufs=3, space="PSUM"))
        ps_eg = ctx.enter_context(tc.tile_pool(name="ps_eg", bufs=3, space="PSUM"))

        ident = sing.tile([P, P], f32)
        make_identity(nc, ident[:])
        sbias = sing.tile([P, 1], f32)     # score bias as AP (Exp needs AP bias)
        nc.vector.memset(sbias[:], SC_BIAS)

        # ---------------- load inputs (emb first, separate DMA queues) ----
        En = sing.tile([P, NBT, D], f32)   # emb rows, tiled by 128
        nc.sync.dma_start(En[:], emb_d[:].rearrange("(t p) d -> p t d", p=P))
        Wn = sing.tile([P, NWT, D], f32)   # weight rows, tiled by 128
        nc.gpsimd.dma_start(Wn[:], w_d[:].rearrange("(t p) d -> p t d", p=P))

        # ---------------- norms (emb before weights) ----------------
        esq = sing.tile([P, NBT], f32)
        for t in range(NBT):
            j = jk.tile([P, D], f32, tag="jact")
            nc.scalar.activation(j[:], En[:, t], AF.Square,
                                 accum_out=esq[:, t : t + 1])
        ne = sing.tile([P, NBT], f32)      # ||emb||
        nc.scalar.activation(ne[:], esq[:], AF.Sqrt)
        ine = sing.tile([P, NBT], f32)     # 1/||emb||
        hine = sing.tile([P, NBT], f32)    # 0.5/||emb||
        nc.vector.reciprocal_approx_accurate(ine[:], ne[:], hine[:])
        nc.vector.tensor_scalar_mul(hine[:], ine[:], 0.5)
        nwsq = sing.tile([P, NWT], f32)    # ||w_row||^2, row-tiled layout
        for t in range(NWT):
            j = jk.tile([P, D], f32, tag="jact")
            nc.scalar.activation(j[:], Wn[:, t], AF.Square,
                                 accum_out=nwsq[:, t : t + 1])

        # nw broadcast rows: roundtrip through DRAM to reorder + partition-bcast
        scr = dram.tile([CS], f32)
        nc.sync.dma_start(scr[:].rearrange("(t p) -> p t", p=P), nwsq[:])
        scr_bc = bass.AP(
            tensor=scr[:].tensor, offset=scr[:].offset,
            ap=[[0, P]] + list(scr[:].ap),
        )
        NWB = sing.tile([P, CS], f32)      # ||w_row|| broadcast over partitions
        nc.sync.dma_start(NWB[:], scr_bc)
        nc.scalar.activation(NWB[:], NWB[:], AF.Sqrt)

        # ---------------- transposed operands ----------------
        # normalize anchor rows first (per-partition scale), then transpose,
        # so VT chunks become available early for the first batch tile.
        nw_row = sing.tile([P, NWT], f32)   # ||w_row||, row-tiled
        inw_row = sing.tile([P, NWT], f32)  # 1/||w_row||
        inw_scr = sing.tile([P, NWT], f32)
        nc.scalar.activation(nw_row[:], nwsq[:], AF.Sqrt)
        nc.vector.reciprocal_approx_accurate(inw_row[:], nw_row[:], inw_scr[:])

        embT = sing.tile([P, B], f32)      # emb^T [d, b]
        for t in range(NBT):
            pst = ps_tr.tile([P, 2 * P], f32, tag="tr")
            nc.tensor.transpose(pst[:, :P], En[:, t], ident[:])
            nc.scalar.copy(embT[:, t * P : (t + 1) * P], pst[:, :P])

        VT = sing.tile([P, CS], f32)       # normalized anchors transposed
        Vn = sing.tile([P, NWT, D], f32)
        for t in range(NWT):
            nc.vector.tensor_scalar_mul(Vn[:, t], Wn[:, t],
                                        inw_row[:, t : t + 1])
            pst = ps_tr.tile([P, 2 * P], f32, tag="tr")
            nc.tensor.transpose(pst[:, :P], Vn[:, t], ident[:])
            nc.scalar.copy(VT[:, t * P : (t + 1) * P], pst[:, :P])



        # persistent per-tile outputs for the batched tail
        d2zall = sing.tile([P, NBT, C_LOC], f32)
        np2zall = sing.tile([P, NBT, C_LOC], f32)

        def build_gram():
            # per-class raw Gram matrices G_c = W_c W_c^T [64, 64], packed
            # into block-diagonal pair matrices GP[:, q*128:(q+1)*128] =
            # [[G_2q, 0], [0, G_2q+1]] so one full-size (0,0)-quadrant matmul
            # computes EG for a transposed class pair (quadrant matmuls
            # interleaved with transposes crash the device).
            WT = sing.tile([P, CS], f32)   # raw W^T = VT * ||w||, gram only
            nc.vector.tensor_mul(WT[:], VT[:], NWB[:])
            Gtmp = sing.tile([S, CS], f32)
            for c in range(C_LOC):
                cs = slice(c * S, (c + 1) * S)
                psg = ps_tr.tile([P, 2 * P], f32, tag="tr")
                nc.tensor.matmul(psg[:S, :S], WT[:, cs], WT[:, cs])
                nc.scalar.copy(Gtmp[:, cs], psg[:S, :S])
            GP = sing.tile([P, CS], f32)
            nc.vector.memset(GP[:], 0.0)
            gt3 = Gtmp[:].rearrange("p (q j) -> p q j", j=2 * S)
            gp3 = GP[:].rearrange("p (q j) -> p q j", j=2 * S)
            nc.sync.dma_start(gp3[0:S, :, 0:S], gt3[:, :, 0:S])
            nc.sync.dma_start(gp3[S : 2 * S, :, S : 2 * S], gt3[:, :, S : 2 * S])
            return GP

        tiles = {}

        def stageA(bt):
            bsl = slice(bt * P, (bt + 1) * P)
            exps = work.tile([P, CS], f32, tag="exps", bufs=2)
            dotr = work.tile([P, CS], f32, tag="dotr", bufs=2)
            for j in range(CS // 512):
                js = slice(j * 512, (j + 1) * 512)
                dotn = ps_mm.tile([P, 512], f32, tag="mm")
                nc.tensor.matmul(dotn[:], embT[:, bsl], VT[:, js])
                nc.scalar.activation(
                    exps[:, js], dotn[:], AF.Exp,
                    bias=sbias[:], scale=hine[:, bt : bt + 1],
                )
                nc.vector.tensor_mul(dotr[:, js], dotn[:], NWB[:, js])

            # top-8 selection per class: R = exps with top8 zeroed
            R = work.tile([P, CS], f32, tag="R", bufs=2)
            for c in range(C_LOC):
                cs = slice(c * S, (c + 1) * S)
                mx8 = small.tile([P, 8], f32, tag="mx8")
                nc.vector.max(out=mx8[:], in_=exps[:, cs])
                nc.vector.match_replace(
                    out=R[:, cs], in_to_replace=mx8[:],
                    in_values=exps[:, cs], imm_value=0.0,
                )
            E = work.tile([P, CS], f32, tag="E", bufs=5)
            nc.gpsimd.tensor_sub(E[:], exps[:], R[:])
            prod_d = work.tile([P, CS], f32, tag="pd", bufs=2)
            nc.gpsimd.tensor_mul(prod_d[:], E[:], dotr[:])
            tiles[bt] = (E, prod_d)

        def stageB(bt, GP):
            E, prod_d = tiles.pop(bt)
            nc.vector.tensor_reduce(
                d2zall[:, bt], prod_d[:].rearrange("p (c s) -> p c s", c=C_LOC),
                axis=mybir.AxisListType.X, op=ALU.add)
            prod_n = work.tile([P, CS], f32, tag="prod_n", bufs=2)
            for q8 in range(CS // 512):
                pse = ps_eg.tile([P, 512], f32, tag="eg")
                for qq in range(2):
                    pst = ps_tr.tile([P, 2 * P], f32, tag="tr")
                    Fq = fpool.tile([P, 2 * P], f32, tag="F")
                    for h in range(2):
                        q = 4 * q8 + 2 * qq + h
                        qs = slice(q * 128, (q + 1) * 128)
                        nc.tensor.transpose(
                            pst[:, h * 128 : (h + 1) * 128], E[:, qs], ident[:]
                        )
                    nc.scalar.copy(Fq[:], pst[:])
                    for h in range(2):
                        q = 4 * q8 + 2 * qq + h
                        qs = slice(q * 128, (q + 1) * 128)
                        nc.tensor.matmul(
                            pse[:, (2 * qq + h) * 128 : (2 * qq + h + 1) * 128],
                            Fq[:, h * 128 : (h + 1) * 128], GP[:, qs],
                        )
                nc.vector.tensor_mul(
                    prod_n[:, q8 * 512 : (q8 + 1) * 512],
                    pse[:], E[:, q8 * 512 : (q8 + 1) * 512],
                )
            nc.vector.tensor_reduce(
                np2zall[:, bt], prod_n[:].rearrange("p (c s) -> p c s", c=C_LOC),
                axis=mybir.AxisListType.X, op=ALU.add)

        # ---------------- software-pipelined main loop (skew 3) ----------
        stageA(0)
        stageA(1)
        GP = build_gram()
        stageA(2)
        stageA(3)
        for bt in range(4, NBT):
            stageB(bt - 4, GP)
            stageA(bt)
        for bt in range(NBT - 4, NBT):
            stageB(bt, GP)

        # ---------------- batched tail ----------------
        # cos2 = d2z * ine / sqrt(np2z);  out = 5*cos2 + 5 + 1e-7
        nps = sing.tile([P, NBT, C_LOC], f32)
        nc.scalar.activation(nps[:], np2zall[:], AF.Sqrt)
        rnp = sing.tile([P, NBT, C_LOC], f32)
        c2 = sing.tile([P, NBT, C_LOC], f32)
        nc.vector.reciprocal_approx_accurate(rnp[:], nps[:], c2[:])
        nc.vector.tensor_mul(c2[:], d2zall[:], rnp[:])
        ine_b = ine[:, :, None].to_broadcast([P, NBT, C_LOC])
        nc.vector.tensor_mul(c2[:], c2[:], ine_b)
        osb = sing.tile([P, NBT, C_LOC], f32)
        nc.vector.tensor_scalar(
            osb[:], c2[:], OUT_SCALE, OUT_BIAS, op0=ALU.mult, op1=ALU.add)
        nc.sync.dma_start(out_d[:].rearrange("(t p) c -> p t c", p=P), osb[:])

    nc.compile()
    return nc


def _get_nc():
    if "nc" not in _CACHE:
        _CACHE["nc"] = build_nc()
    return _CACHE["nc"]


def kernel(emb: np.ndarray, weight: np.ndarray) -> np.ndarray:
    from concourse.bass_utils import run_bass_kernel_spmd

    emb = np.ascontiguousarray(np.asarray(emb, dtype=np.float32))
    weight = np.ascontiguousarray(np.asarray(weight, dtype=np.float32))
    assert emb.shape == (B, D) and weight.shape == (C, S, D)

    nc = _get_nc()
    in_maps = [
        {
            "emb": emb,
            "weight": np.ascontiguousarray(
                weight[i * C_LOC : (i + 1) * C_LOC].reshape(CS, D)
            ),
        }
        for i in range(NCORES)
    ]
    res = run_bass_kernel_spmd(nc, in_maps, core_ids=list(range(NCORES)))
    return np.concatenate(
        [res.results[i]["out"] for i in range(NCORES)], axis=1
    )
